# revision 1
# baseline (speedup 1.0000x reference)
"""Trainium2 Bass kernel for causal multi-head attention.

Problem: nn_MultiHeadAttention (B=4, N=2048, D=768, H=12, dh=64), fp32 I/O.

Sharding: 8 cores = 4 batches x 2 head-groups (6 heads each).  Each core
computes QKV projections for its 6 heads, causal softmax attention, and a
partial output projection (its heads' rows of Wo).  The two partials per
batch are summed on the host (tensor-parallel reduce); the bias is added on
the hg=0 core only.

Per-core layout strategy (all matmuls in bf16, fp32 accumulate):
  - X^T is prepared host-side: xt[c,p,n] = X[n, 128c+p] (bf16).
  - Q^T, K^T computed as [384, N] (d_out on partitions) directly.
  - V computed in natural [N, 64h] layout, extended with a ones column per
    head so the context matmul also produces the softmax denominators.
  - scores^T tiles [k=128, 2 heads, q=512] in PSUM, exp on ScalarE
    (scale=1/8 fused), causal diag masked by memset + triangle multiply.
  - ctx^T accumulated in PSUM over k chunks; row 64 = sum_k exp (denom).
  - denominators -> reciprocal on a [128,8]-packed tile -> DRAM ->
    partition-broadcast DMA -> per-q inverse scale applied on DVE.
  - out = ctxn^T.T @ Wo + bias, bias fused into the PSUM->SBUF copy.
"""

import sys

sys.path.insert(0, "/opt/trn_rl_repo")

import numpy as np
import ml_dtypes

BF16 = ml_dtypes.bfloat16

P = 128
DIN = 768
DH = 384  # per-core output cols of Wq/Wk/Wv (6 heads x 64)
NH = 6  # heads per core
KCH = 6  # d_in chunks (768/128)
QW = 512  # q block width


def build(seq=2048, n_wchunks=3):
    """Build the SPMD single-core program.  seq parameterized for sim tests."""
    import concourse.mybir as mybir
    import concourse.tile as tile
    from concourse import bacc
    from contextlib import ExitStack

    f32 = mybir.dt.float32
    bf16 = mybir.dt.bfloat16
    EXP = mybir.ActivationFunctionType.Exp
    MULT = mybir.AluOpType.mult
    ADD = mybir.AluOpType.add

    nqb = seq // QW  # q blocks of 512
    nkc = seq // P  # k chunks of 128
    nqs = seq // P  # out row chunks of 128
    HP = 3  # head pairs

    nc = bacc.Bacc(None, target_bir_lowering=False, debug=False)

    xt_d = nc.dram_tensor("xt", [KCH, P, seq], bf16, kind="ExternalInput")
    wq_d = nc.dram_tensor("wq", [KCH, P, DH], bf16, kind="ExternalInput")
    wk_d = nc.dram_tensor("wk", [KCH, P, DH], bf16, kind="ExternalInput")
    wv_d = nc.dram_tensor("wv", [KCH, P, DH], bf16, kind="ExternalInput")
    wo_d = nc.dram_tensor("wo", [n_wchunks, P, DIN], bf16, kind="ExternalInput")
    bias_d = nc.dram_tensor("bias", [P, DIN // P], f32, kind="ExternalInput")
    tri_d = nc.dram_tensor("tri", [P, P], bf16, kind="ExternalInput")
    # output is stored transposed: out[e_chunk, e_p, q] = full_out[q, 128*e_chunk+e_p]
    out_d = nc.dram_tensor("out", [DIN // P, P, seq], f32, kind="ExternalOutput")
    inv_d = nc.dram_tensor("inv_scratch", [HP, nqb, 2, QW], f32)

    with tile.TileContext(nc) as tc, ExitStack() as ctx:
        const = ctx.enter_context(tc.tile_pool(name="const", bufs=1))
        io = ctx.enter_context(tc.tile_pool(name="io", bufs=1))
        expp = ctx.enter_context(tc.tile_pool(name="expp", bufs=8))
        crawp = ctx.enter_context(tc.tile_pool(name="crawp", bufs=3))
        smallp = ctx.enter_context(tc.tile_pool(name="smallp", bufs=4))
        invbp = ctx.enter_context(tc.tile_pool(name="invbp", bufs=4))
        outp = ctx.enter_context(tc.tile_pool(name="outp", bufs=3))
        ps = ctx.enter_context(tc.tile_pool(name="ps", bufs=3, space="PSUM"))
        cxps = ctx.enter_context(tc.tile_pool(name="cxps", bufs=1, space="PSUM"))

        # ---------------- persistent inputs ----------------
        xt = const.tile([P, KCH, seq], bf16, name="xt_sb")
        wq = const.tile([P, KCH, DH], bf16, name="wq_sb")
        wk = const.tile([P, KCH, DH], bf16, name="wk_sb")
        wv = const.tile([P, KCH, DH], bf16, name="wv_sb")
        wo = const.tile([P, n_wchunks, DIN], bf16, name="wo_sb")
        bias = const.tile([P, DIN // P], f32, name="bias_sb")
        tri = const.tile([P, P], bf16, name="tri_sb")
        # inputs: activations stream on the sync HWDGE queue, weights in
        # parallel on the scalar HWDGE queue (ScalarE is idle at startup)
        for c in range(KCH):
            nc.scalar.dma_start(wq[:, c, :], wq_d[c])
            nc.sync.dma_start(xt[:, c, :], xt_d[c])
        for c in range(KCH):
            nc.scalar.dma_start(wk[:, c, :], wk_d[c])
        nc.scalar.dma_start(tri[:], tri_d[:])
        for c in range(KCH):
            nc.scalar.dma_start(wv[:, c, :], wv_d[c])
        for c in range(n_wchunks):
            nc.scalar.dma_start(wo[:, c, :], wo_d[c])
        nc.scalar.dma_start(bias[:], bias_d[:])

        # persistent activations
        qt = io.tile([P, HP, seq], bf16, name="qt_sb")
        kt = io.tile([P, HP, seq], bf16, name="kt_sb")
        vx = io.tile([P, nkc, NH, 65], bf16, name="vx_sb")
        cn = io.tile([P, HP, seq], bf16, name="cn_sb")
        nc.vector.memset(vx[:, :, :, 64:65], 1.0)

        def qk_quarter(pair, quarter):
            """Project one quarter of pair's Q^T/K^T: one weight chunk reused
            across two 512-wide n blocks (kc-outer keeps LDWEIGHTS warm).
            Yields after each matmul so the caller can interleave."""
            wt, dst = (wq, qt) if quarter < 2 else (wk, kt)
            nbs = (0, 1) if quarter % 2 == 0 else (2, 3)
            if nbs[-1] >= nqb:  # small-seq (sim) builds
                nbs = tuple(nb for nb in nbs if nb < nqb)
                if not nbs:
                    return
            pt = ps.tile([P, 2, QW], f32, tag="quad", name="pt")
            for kc in range(KCH):
                for r, nb in enumerate(nbs):
                    nc.tensor.matmul(
                        pt[:, r, :],
                        lhsT=wt[:, kc, pair * P : (pair + 1) * P],
                        rhs=xt[:, kc, nb * QW : (nb + 1) * QW],
                        start=(kc == 0),
                        stop=(kc == KCH - 1),
                    )
                    yield
            for r, nb in enumerate(nbs):
                nc.vector.tensor_copy(dst[:, pair, nb * QW : (nb + 1) * QW], pt[:, r, :])

        def qk_upfront():
            """Pair-0 Q^T and K^T over the first two n blocks, interleaved by
            k chunk so compute follows the xt DMA stream."""
            nbs = tuple(nb for nb in (0, 1) if nb < nqb)
            ptq = ps.tile([P, 2, QW], f32, tag="quad", name="ptq")
            ptk = ps.tile([P, 2, QW], f32, tag="quad", name="ptk")
            for kc in range(KCH):
                for pt, wt in ((ptq, wq), (ptk, wk)):
                    for r, nb in enumerate(nbs):
                        nc.tensor.matmul(
                            pt[:, r, :],
                            lhsT=wt[:, kc, 0:P],
                            rhs=xt[:, kc, nb * QW : (nb + 1) * QW],
                            start=(kc == 0),
                            stop=(kc == KCH - 1),
                        )
            for pt, dst in ((ptq, qt), (ptk, kt)):
                for r, nb in enumerate(nbs):
                    nc.vector.tensor_copy(dst[:, 0, nb * QW : (nb + 1) * QW], pt[:, r, :])

        def v_chunk(nb):
            """Yields after each matmul so the caller can interleave."""
            pt = ps.tile([P, 2, QW], f32, tag="quad", name="pt")
            for kc in range(KCH):
                nc.tensor.matmul(
                    pt[:, 0, :DH],
                    lhsT=xt[:, kc, nb * P : (nb + 1) * P],
                    rhs=wv[:, kc, :],
                    start=(kc == 0),
                    stop=(kc == KCH - 1),
                )
                yield
            nc.vector.tensor_copy(
                vx[:, nb, :, 0:64],
                pt[:, 0, :DH].rearrange("p (h d) -> p h d", d=64),
            )

        def out_proj_t(e, qb):
            """Transposed output projection: out^T[e-chunk, q-block] =
            Wo_chunk^T @ cn, bias as a per-partition scalar.  Yields after
            each matmul so the caller can interleave."""
            qsl = slice(qb * QW, (qb + 1) * QW)
            op = ps.tile([P, 2, QW], f32, tag="quad", name="op")
            for c in range(n_wchunks):
                nc.tensor.matmul(
                    op[:, 0, :],
                    lhsT=wo[:, c, e * P : (e + 1) * P],
                    rhs=cn[:, c, qsl],
                    start=(c == 0),
                    stop=(c == n_wchunks - 1),
                )
                yield
            ob = outp.tile([P, QW], f32, name="ob")
            nc.vector.tensor_scalar_add(ob[:], op[:, 0, :], bias[:, e : e + 1])
            nc.sync.dma_start(out_d[e, :, qsl], ob[:])

        class FillQueue:
            """Queue of instruction generators, driven a few matmuls at a
            time from inside the attention loop to fill PE bubbles."""

            def __init__(self):
                self.gens = []  # (label, gen)

            def add(self, gen, label=None):
                self.gens.append((label, gen))

            def step(self, n):
                while n > 0 and self.gens:
                    try:
                        next(self.gens[0][1])
                        n -= 1
                    except StopIteration:
                        self.gens.pop(0)

            def drain_through(self, label):
                while any(lab == label for lab, _ in self.gens):
                    try:
                        next(self.gens[0][1])
                    except StopIteration:
                        self.gens.pop(0)

            def drain(self):
                while self.gens:
                    self.step(1000)

        def attention_qblock(pair, i, fq, drain_label=None, steps=(1, 2)):
            """One 512-wide q block of causal attention for a head pair.
            fq: FillQueue driven mid-loop to fill PE bubbles."""
            nj = 4 * i + 4  # active k chunks
            qsl = slice(i * QW, (i + 1) * QW)
            cx = cxps.tile([65, 2, QW], f32, name="cx")
            pend = []  # (j, exp tile) awaiting ctx matmuls

            def ctx_mms(j, et):
                d = j - 4 * i
                off = P * d if d > 0 else 0
                for hh in (0, 1):
                    nc.tensor.matmul(
                        cx[:, hh, off:],
                        lhsT=vx[:, j, 2 * pair + hh, :],
                        rhs=et[:, hh, off:],
                        start=(j == 0),
                        stop=(j == nj - 1),
                    )

            for j in range(nj):
                d = j - 4 * i
                off = P * d if d > 0 else 0
                sc = ps.tile([P, 2, QW], f32, tag="quad", name="sc")
                for hh in (0, 1):
                    base = 64 * hh
                    nc.tensor.matmul(
                        sc[:, hh, off:],
                        lhsT=kt[base : base + 64, pair, j * P : (j + 1) * P],
                        rhs=qt[base : base + 64, pair, i * QW + off : (i + 1) * QW],
                        start=True,
                        stop=True,
                        tile_position=(base, 0),
                    )
                et = expp.tile([P, 2, QW], bf16, name="et")
                if off:
                    nc.scalar.activation(
                        et[:, :, off:], sc[:, :, off:], EXP, scale=0.125
                    )
                else:
                    nc.scalar.activation(et[:], sc[:], EXP, scale=0.125)
                if d >= 0:
                    for hh in (0, 1):
                        if off:
                            nc.vector.memset(et[:, hh, 0:off], 0.0)
                        nc.vector.tensor_mul(
                            et[:, hh, off : off + P], et[:, hh, off : off + P], tri[:]
                        )
                if len(pend) >= 2:
                    ctx_mms(*pend.pop(0))
                pend.append((j, et))
                fq.step(steps[1] if d >= 0 else steps[0])
            for args in pend:
                ctx_mms(*args)
            if drain_label is not None:
                fq.drain_through(drain_label)

            # ctx + denominators out of PSUM
            cr = crawp.tile([65, 2, QW], f32, name="cr")
            nc.vector.tensor_copy(cr[:], cx[:])

            # denominators -> packed reciprocal -> DRAM -> broadcast
            sp = smallp.tile([P, 8], f32, tag="sp", name="sp")
            nc.sync.dma_start(sp[:], cr[64:65, :, :])
            ip = smallp.tile([P, 8], f32, tag="ip", name="ip")
            nc.vector.reciprocal(ip[:], sp[:])
            nc.sync.dma_start(inv_d[pair, i], ip[:])

            for hh in (0, 1):
                ib = invbp.tile([64, QW], f32, name="ib")
                nc.sync.dma_start(
                    ib[:], inv_d[pair, i, hh : hh + 1, :].broadcast_to((64, QW))
                )
                if hh == 0:
                    nc.vector.tensor_mul(cn[0:64, pair, qsl], cr[0:64, 0, :], ib[:])
                else:
                    tt = smallp.tile([64, QW], bf16, tag="tt", name="tt")
                    nc.vector.tensor_mul(tt[:], cr[0:64, 1, :], ib[:])
                    nc.sync.dma_start(cn[64:128, pair, qsl], tt[:])

        # ---------------- schedule ----------------
        # Dense upfront phases (keeps the PE clock-gate warm), then
        # attention with later pairs' projections / the output projection
        # interleaved as fine-grained fill work.
        for quarter in range(4):
            for _ in qk_quarter(0, quarter):
                pass
        for nb in range(nkc):
            for _ in v_chunk(nb):
                pass
        for pair in range(HP):
            last_pair = pair == HP - 1
            fq = FillQueue()
            if not last_pair:
                for quarter in range(4):
                    fq.add(qk_quarter(pair + 1, quarter), f"qk{pair + 1}")
            for i in range(nqb):
                attention_qblock(pair, i, fq, steps=(1, 2))
                if last_pair:
                    for e in range(DIN // P):
                        fq.add(out_proj_t(e, i), "op")
            fq.drain()

    nc.compile()
    return nc


def make_in_maps(X, Wq, Wk, Wv, Wo, bo, seq=2048):
    """Shard full inputs into the 8 per-core input maps."""
    X = np.asarray(X, np.float32)
    Wq = np.asarray(Wq, np.float32)
    Wk = np.asarray(Wk, np.float32)
    Wv = np.asarray(Wv, np.float32)
    Wo = np.asarray(Wo, np.float32)
    bo = np.asarray(bo, np.float32)

    tri = np.triu(np.ones((P, P), np.float32)).astype(BF16)
    bias_full = np.ascontiguousarray(bo.reshape(DIN // P, P).T).astype(np.float32)
    bias_zero = np.zeros((P, DIN // P), np.float32)

    in_maps = []
    for b in range(X.shape[0]):
        xt = np.ascontiguousarray(X[b].T).astype(BF16).reshape(KCH, P, seq)
        for hg in range(2):
            sl = slice(hg * DH, (hg + 1) * DH)
            in_maps.append(
                {
                    "xt": xt,
                    "wq": np.ascontiguousarray(Wq[:, sl]).astype(BF16).reshape(KCH, P, DH),
                    "wk": np.ascontiguousarray(Wk[:, sl]).astype(BF16).reshape(KCH, P, DH),
                    "wv": np.ascontiguousarray(Wv[:, sl]).astype(BF16).reshape(KCH, P, DH),
                    "wo": np.ascontiguousarray(Wo[sl, :]).astype(BF16).reshape(3, P, DIN),
                    "bias": bias_full if hg == 0 else bias_zero,
                    "tri": tri,
                }
            )
    return in_maps


_built = None


def _get_built():
    global _built
    if _built is None:
        _built = build()
    return _built


def run(inputs, trace=False):
    from concourse.bass_utils import run_bass_kernel_spmd

    nc = _get_built()
    in_maps = make_in_maps(**inputs)
    res = run_bass_kernel_spmd(nc, in_maps, list(range(8)), trace=trace)
    # per-core output is stored transposed as [6, 128, seq] = out.T chunked
    parts = [
        np.asarray(r["out"], np.float32).reshape(DIN, -1).T for r in res.results
    ]
    out = np.stack([parts[2 * b] + parts[2 * b + 1] for b in range(len(parts) // 2)])
    return out, res


def kernel(X, Wq, Wk, Wv, Wo, bo):
    out, _ = run(dict(X=X, Wq=Wq, Wk=Wk, Wv=Wv, Wo=Wo, bo=bo))
    return out

